# revision 1
# baseline (speedup 1.0000x reference)
"""Trainium2 8-core SPMD kernel for MQA attention with relative position bias.

Reference computation (b=2, n=2048, D=1024, h=8, dh=64, MQA single k/v head):
    q  = x @ Wq;  kv = x @ Wkv;  k, v = kv[..., :64], kv[..., 64:]
    sim = (q[b,h,i,:] . k[b,j,:]) * dh**-0.5 + rel_pos_bias[h,i,j]   (causal masked)
    out = softmax(sim) @ v  -> reshape -> @ Wo + bo

Sharding: queries are sharded across the 8 cores. Core c owns q-tiles
{c, 15-c} of each batch (128 tokens per tile -> 512 tokens/core), which
balances causal work exactly (each core runs 8+16 = 24 (slot, j-tile)
pairs per batch, pads are fully masked via the bias input). All 8 heads
are computed on every core (MQA: single shared k/v head). The k/v
projection is token-sharded and exchanged with one small bf16 AllGather.
The output projection is row-sharded (each core emits its own 512 token
rows) so no output collective is needed.

On-device layout notes:
  - Scores are computed transposed (S^T: keys on partitions, queries on
    free dim) so the softmax denominator comes from a ones-column in V
    (M=65 PV matmuls) instead of a partition reduction.
  - rel_pos_bias is pre-transposed/masked/scaled(x8) on the host, cast
    to bf16, and injected into PSUM with a bf16 identity matmul; the
    softmax scale 1/8 is applied inside the ScalarE exp activation.
  - Projections run as float32r (full-rate fp32 on the PE at N>=256),
    attention matmuls run bf16.
"""

import os
import sys

import numpy as np

sys.path.insert(0, "/opt/trn_rl_repo")

import ml_dtypes

BF16 = ml_dtypes.bfloat16

# ---- problem constants (hardcoded per the harness contract) ----
B = 2
N = 2048
DIM = 1024
HEADS = 8
DH = 64
INNER = HEADS * DH  # 512
P = 128
NT = N // P  # 16 q/k tiles per batch
EXTA, EXTB = 8, 16  # j-tile extents for slot A (q-tile c) / slot B (q-tile 15-c)
NPAIR = EXTA + EXTB  # 24 (slot, j-tile) pairs per batch per core
NCORES = 8
TOK_OWN = 4 * P  # 512 own tokens per core
NEG = -1.0e30  # masked logit (pre-scale), exp -> 0

_CACHE = {}


def _q_tiles(c):
    return [c, NT - 1 - c]


def _placement():
    """global (batch, tile) -> (rank, slot) in the AllGather layout."""
    m = {}
    for c in range(NCORES):
        for b in range(B):
            for sl, t in enumerate(_q_tiles(c)):
                m[(b, t)] = (c, 2 * b + sl)
    return m


def build_graph(recip_fast=True, use_pbcast=True, use_liblod=False, use_cc=True, use_dvedup=True, use_f32r=True, phase=0, pack=True, use_exp=True, pv65=True, reps=1):
    import concourse.bass as bass
    import concourse.bacc as bacc
    import concourse.mybir as mybir
    import concourse.tile as tile
    from concourse import library_config

    dt = mybir.dt
    f32, f32r, bf16 = dt.float32, dt.float32r, dt.bfloat16
    if not use_f32r:
        f32r = dt.float32
    AF = mybir.ActivationFunctionType

    nc = bacc.Bacc(None, target_bir_lowering=False)

    # ---- I/O ----
    xT_t = nc.dram_tensor("xT", [DIM, TOK_OWN], f32r, kind="ExternalInput")
    Wq_t = nc.dram_tensor("Wq", [DIM, INNER], f32r, kind="ExternalInput")
    Wk_t = nc.dram_tensor("Wk", [DIM, DH], f32r, kind="ExternalInput")
    Wv_t = nc.dram_tensor("Wv", [DIM, DH], f32r, kind="ExternalInput")
    Wo_t = nc.dram_tensor("Wo", [INNER, DIM], bf16, kind="ExternalInput")
    bo_t = nc.dram_tensor("bo", [1, DIM], f32r, kind="ExternalInput")
    ident_t = nc.dram_tensor("ident", [P, P], bf16, kind="ExternalInput")
    ones_r_t = nc.dram_tensor("ones_r", [1, P], f32r, kind="ExternalInput")
    ones_b_t = nc.dram_tensor("ones_b", [P, 1], bf16, kind="ExternalInput")
    # biasT[b, pair, j, h, q]: transposed, causal-masked, x8-scaled bias
    bias_t = nc.dram_tensor(
        "biasT", [B, NPAIR, P, HEADS, P], bf16, kind="ExternalInput"
    )
    out_t = nc.dram_tensor("out", [TOK_OWN, DIM], f32, kind="ExternalOutput")

    plc = _placement()

    with tile.TileContext(nc) as tc:
        with (
            tc.tile_pool(name="const", bufs=1) as cpool,
            tc.tile_pool(name="bias", bufs=3) as bpool,
            tc.tile_pool(name="pt", bufs=3) as ptpool,
            tc.tile_pool(name="at", bufs=2) as atpool,
            tc.tile_pool(name="ob", bufs=2) as obpool,
            tc.tile_pool(name="ps", bufs=2, space="PSUM") as pspool,
            tc.tile_pool(name="dram", bufs=1, space="DRAM") as dpool,
        )\
        :
            # ---- constants / weights into SBUF ----
            xT_sb = cpool.tile([P, 8 * TOK_OWN], f32r, tag="xT_sb")  # [128, 8 chunks x 512]
            for fc in range(8):
                nc.sync.dma_start(
                    out=xT_sb[:, fc * TOK_OWN : (fc + 1) * TOK_OWN],
                    in_=xT_t[fc * P : (fc + 1) * P, :],
                )
            Wq_sb = cpool.tile([P, 8 * INNER], f32r, tag="Wq_sb")
            for fc in range(8):
                nc.sync.dma_start(
                    out=Wq_sb[:, fc * INNER : (fc + 1) * INNER],
                    in_=Wq_t[fc * P : (fc + 1) * P, :],
                )
            Wk_sb = cpool.tile([P, 8 * DH], f32r, tag="Wk_sb")
            for fc in range(8):
                nc.sync.dma_start(
                    out=Wk_sb[:, fc * DH : (fc + 1) * DH],
                    in_=Wk_t[fc * P : (fc + 1) * P, :],
                )
            Wv_sb = cpool.tile([P, 8 * DH], f32r, tag="Wv_sb")
            for fc in range(8):
                nc.sync.dma_start(
                    out=Wv_sb[:, fc * DH : (fc + 1) * DH],
                    in_=Wv_t[fc * P : (fc + 1) * P, :],
                )
            Wo_sb = cpool.tile([P, 4 * DIM], bf16, tag="Wo_sb")  # chunk hp at cols hp*1024
            for fc in range(4):
                nc.sync.dma_start(
                    out=Wo_sb[:, fc * DIM : (fc + 1) * DIM],
                    in_=Wo_t[fc * P : (fc + 1) * P, :],
                )
            bo_sb = cpool.tile([1, DIM], f32r, tag="bo_sb")
            nc.sync.dma_start(out=bo_sb[:], in_=bo_t[:])
            ident_sb = cpool.tile([P, P], bf16, tag="ident_sb")
            nc.sync.dma_start(out=ident_sb[:], in_=ident_t[:])
            ones128 = cpool.tile([1, P], f32r, tag="ones128")
            nc.sync.dma_start(out=ones128[:], in_=ones_r_t[:])

            # ---- k/v projection on own tokens, staged for AllGather ----
            # staging[0:64, 0:512]   = kT_own (bf16), rows 64:128 duplicate
            # staging[:, 512:772]    = V65_own ([tok,64] + ones col per tile)
            staging = cpool.tile([P, INNER + 4 * (DH + 1)], bf16, tag="staging")  # [128, 772]
            kps = pspool.tile([P, TOK_OWN], f32, tag="sT")
            for fc in range(8):
                nc.tensor.matmul(
                    kps[0:DH, :],
                    Wk_sb[:, fc * DH : (fc + 1) * DH],
                    xT_sb[:, fc * TOK_OWN : (fc + 1) * TOK_OWN],
                    start=(fc == 0),
                    stop=(fc == 7),
                )
            nc.vector.tensor_copy(staging[0:DH, 0:TOK_OWN], kps[0:DH, :])
            if use_dvedup:
                nc.vector.tensor_copy(staging[DH:P, 0:TOK_OWN], kps[0:DH, :])
            else:
                nc.sync.dma_start(
                    out=staging[DH:P, 0:TOK_OWN], in_=staging[0:DH, 0:TOK_OWN]
                )
            for tt in range(4):
                vps = pspool.tile([P, DH], f32, tag="sT", name=f"vps{tt}")
                for fc in range(8):
                    nc.tensor.matmul(
                        vps[:, :],
                        xT_sb[:, fc * TOK_OWN + tt * P : fc * TOK_OWN + (tt + 1) * P],
                        Wv_sb[:, fc * DH : (fc + 1) * DH],
                        start=(fc == 0),
                        stop=(fc == 7),
                    )
                col = INNER + tt * (DH + 1)
                nc.vector.tensor_copy(staging[:, col : col + DH], vps[:, :])
                nc.sync.dma_start(
                    out=staging[:, col + DH : col + DH + 1], in_=ones_b_t[:]
                )

            bounce = dpool.tile([P, 772], bf16)
            gathered = dpool.tile([NCORES * P, 772], bf16, addr_space="Shared")
            nc.sync.dma_start(out=bounce[:], in_=staging[:])
            if use_cc:
                nc.gpsimd.collective_compute(
                    "AllGather",
                    mybir.AluOpType.bypass,
                    replica_groups=[list(range(NCORES))],
                    ins=[bounce[:].opt()],
                    outs=[gathered[:].opt()],
                )
            else:
                for r in range(NCORES):
                    nc.sync.dma_start(
                        out=gathered[r * P : (r + 1) * P, :], in_=bounce[:]
                    )
            # partition_broadcast (normalize) needs the 'attn' gpsimd library
            if use_liblod:
                nc.gpsimd.load_library(library_config.attn)

            # ---- q projection (overlaps the AllGather) ----
            # head-major layout [64, h*512 + tok] so one scores matmul can
            # stream two heads of q against the shared MQA k (3D rhs AP)
            qT_sb = cpool.tile([DH, HEADS * TOK_OWN], bf16, tag="qT_sb")
            for hp in range(4):
                qps = pspool.tile([P, TOK_OWN], f32, tag="sT", name=f"qps{hp}")
                for fc in range(8):
                    nc.tensor.matmul(
                        qps[:, :],
                        Wq_sb[:, fc * INNER + hp * P : fc * INNER + (hp + 1) * P],
                        xT_sb[:, fc * TOK_OWN : (fc + 1) * TOK_OWN],
                        start=(fc == 0),
                        stop=(fc == 7),
                    )
                nc.vector.tensor_copy(
                    qT_sb[0:DH, (2 * hp) * TOK_OWN : (2 * hp + 1) * TOK_OWN],
                    qps[0:DH, :],
                )
                nc.vector.tensor_copy(
                    qT_sb[0:DH, (2 * hp + 1) * TOK_OWN : (2 * hp + 2) * TOK_OWN],
                    qps[DH:P, :],
                )

            if phase == 5:
                nc.gpsimd.dma_start(out=out_t[0:P, :], in_=xT_sb[:, 0:1024])
                nc.gpsimd.dma_start(out=out_t[P : 2 * P, :], in_=Wq_sb[:, 0:1024])
            if phase == 4:
                for r4 in range(4):
                    nc.gpsimd.dma_start(
                        out=out_t[r4 * DH : (r4 + 1) * DH, :],
                        in_=qT_sb[0:DH, r4 * 1024 : (r4 + 1) * 1024],
                    )
            # ---- load gathered k (duplicated rows) and V65 tiles ----
            kT2 = cpool.tile([P, B * N], bf16, tag="kT2")  # (b, jt) at cols b*2048 + jt*128
            V65 = cpool.tile([P, B * NT * (DH + 1)], bf16, tag="V65")  # (b,jt) at *(65)
            for b in range(B):
                for jt in range(NT):
                    r, s = plc[(b, jt)]
                    nc.sync.dma_start(
                        out=kT2[:, (b * NT + jt) * P : (b * NT + jt + 1) * P],
                        in_=gathered[r * P : (r + 1) * P, s * P : (s + 1) * P],
                    )
                    g = (b * NT + jt) * (DH + 1)
                    nc.sync.dma_start(
                        out=V65[:, g : g + DH + 1],
                        in_=gathered[
                            r * P : (r + 1) * P,
                            INNER + s * (DH + 1) : INNER + (s + 1) * (DH + 1),
                        ],
                    )

            if phase == 7:
                for c7 in range(4):
                    nc.gpsimd.dma_start(
                        out=out_t[c7 * P : (c7 + 1) * P, :],
                        in_=kT2[:, c7 * 1024 : (c7 + 1) * 1024],
                    )
            # ---- attention + output projection per (batch, slot) ----
            for rep in range(reps):
                if phase == 1:
                    nc.gpsimd.dma_start(
                        out=out_t[0:P, 0:772], in_=staging[:, :]
                    )
                for b in (range(B) if phase in (0, 2, 6, 8) else []):
                    for sl, ext in ((0, EXTA), (1, EXTB)):
                        qcol = (2 * b + sl) * P  # q columns in qT/attnT order
                        pv = pspool.tile(
                            [P, HEADS * P], f32, tag="pv", name=f"pv{b}{sl}"
                        )
                        for jt in range(ext):
                            pair = jt if sl == 0 else EXTA + jt
                            bias_sb = bpool.tile([P, HEADS * P], bf16, tag="bias")
                            nc.sync.dma_start(
                                out=bias_sb[:],
                                in_=bias_t[b, pair]
                                .rearrange("j h q -> j (h q)"),
                            )
                            sT = pspool.tile(
                                [P, HEADS * P], f32, tag="sT", name=f"sT{b}{sl}{jt}"
                            )
                            # bias injection: sT = ident.T @ bias
                            for half in range(2):
                                nc.tensor.matmul(
                                    sT[:, half * 512 : (half + 1) * 512],
                                    ident_sb[:, :],
                                    bias_sb[:, half * 512 : (half + 1) * 512],
                                    start=True,
                                    stop=False,
                                    skip_group_check=True,
                                )
                            # scores: sT[:, h*128:+128] += kT.T @ qT_h
                            kcol = (b * NT + jt) * P
                            qT3 = qT_sb[0:DH, :].rearrange(
                                "p (h t) -> p h t", h=HEADS
                            )
                            for hp in range(4):
                                nc.tensor.matmul(
                                    sT[:, (2 * hp) * P : (2 * hp + 2) * P],
                                    kT2[0:DH, kcol : kcol + P],
                                    qT3[:, 2 * hp : 2 * hp + 2, qcol : qcol + P],
                                    start=False,
                                    stop=True,
                                    skip_group_check=True,
                                )
                            # softmax numerator: P^T = exp(sT/8) in bf16
                            pt_sb = ptpool.tile([P, HEADS * P], bf16, tag="pt")
                            if use_exp:
                                nc.scalar.activation(
                                    pt_sb[:, :], sT[:, :], AF.Exp, scale=0.125
                                )
                            else:
                                nc.vector.tensor_copy(pt_sb[:, :], sT[:, :])
                            if phase == 6 and b == 0 and sl == 0 and jt == 0:
                                nc.gpsimd.dma_start(out=out_t[0:P, :], in_=pt_sb[:, :])
                                s6 = obpool.tile([P, 512], f32, tag="ob", name="s6")
                                nc.vector.tensor_copy(s6[:, :], sT[:, 0:512])
                                nc.sync.dma_start(out=out_t[P : 2 * P, 0:512], in_=s6[:, :])
                            # PV: pv[0:65, h*128:+128] += V65.T @ P^T_h
                            g = (b * NT + jt) * (DH + 1)
                            mdim = DH + 1 if pv65 else DH
                            for h in range(HEADS):
                                nc.tensor.matmul(
                                    pv[0:mdim, h * P : (h + 1) * P],
                                    V65[:, g : g + mdim],
                                    pt_sb[:, h * P : (h + 1) * P],
                                    # one start per 2KB PSUM zero-region (h=0 and
                                    # h=4); other heads' first write lands on
                                    # pending-zero bytes and overwrites correctly
                                    start=(jt == 0 and h % 4 == 0),
                                    stop=(jt == ext - 1),
                                    skip_group_check=True,
                                )

                        if phase == 8:
                            if b == 0 and sl == 1:
                                pr8 = ptpool.tile([P, HEADS * P], f32, tag="pt", name="pr8")
                                nc.vector.tensor_copy(pr8[0 : DH + 1, :], pv[0 : DH + 1, :])
                                nc.sync.dma_start(out=out_t[0 : DH + 1, :], in_=pr8[0 : DH + 1, :])
                            continue
                        if phase in (2, 6):
                            if phase == 6:
                                continue
                            pr_sb = obpool.tile([P, 512], f32, tag="ob", name=f"pr{b}{sl}")
                            nc.vector.tensor_copy(pr_sb[0:DH, :], pv[0:DH, 0:512])
                            nc.sync.dma_start(
                                out=out_t[(2 * b + sl) * P : (2 * b + sl) * P + DH, 0:512],
                                in_=pr_sb[0:DH, :],
                            )
                            continue
                        # ---- normalize: attnT = pv[0:64] * (1/l) ----
                        # 1/l via ACT: exp(-log(l)) — one table set holds both fns
                        recip = cpool.tile([1, HEADS * P], f32, name=f"rc{b}{sl}", tag="recip", bufs=2)
                        lg = cpool.tile([1, HEADS * P], f32, name=f"lg{b}{sl}", tag="lg", bufs=2)
                        nc.scalar.activation(lg[:, :], pv[DH : DH + 1, :], AF.Ln)
                        nc.scalar.activation(recip[:, :], lg[:, :], AF.Exp, scale=-1.0)
                        bc_sb = ptpool.tile(
                            [DH, HEADS * P], f32, tag="pt", name=f"bc{b}{sl}"
                        )
                        if use_pbcast:
                            nc.gpsimd.partition_broadcast(bc_sb[:, :], recip[:, :])
                        attnT = atpool.tile([P, HEADS * P], bf16, tag="at")
                        if not use_pbcast and not recip_fast:
                            nc.vector.tensor_copy(attnT[0:DH, :], pv[0:DH, :])
                        elif use_pbcast:
                            for half in range(2):
                                fs = slice(half * 512, (half + 1) * 512)
                                nc.vector.tensor_mul(attnT[0:DH, fs], pv[0:DH, fs], bc_sb[:, fs])
                        else:
                            nc.vector.tensor_copy(bc_sb[0:1, :], recip[:, :])
                            for half in range(2):
                                fs = slice(half * 512, (half + 1) * 512)
                                nc.vector.tensor_mul(attnT[0:DH, fs], pv[0:DH, fs], bc_sb[:, fs])
                        # shifted duplicate: rows 64:128 col g*128 hold head g+1
                        if use_dvedup:
                            nc.vector.tensor_copy(
                                attnT[DH:P, 0 : 7 * P], attnT[0:DH, P : HEADS * P]
                            )
                        else:
                            nc.sync.dma_start(
                                out=attnT[DH:P, 0 : 7 * P],
                                in_=attnT[0:DH, P : HEADS * P],
                            )

                        # ---- output projection for this slot's 128 tokens ----
                        orow = (2 * b + sl) * P
                        for half in range(2):
                            fs = slice(half * 512, (half + 1) * 512)
                            ops = pspool.tile(
                                [P, 512], f32, tag="pv", name=f"op{b}{sl}{half}"
                            )
                            nc.tensor.matmul(
                                ops[:, :], ones128[:, :], bo_sb[:, fs], start=True, stop=False
                            )
                            for hp in range(4):
                                nc.tensor.matmul(
                                    ops[:, :],
                                    attnT[:, 2 * hp * P : (2 * hp + 1) * P],
                                    Wo_sb[:, hp * DIM + half * 512 : hp * DIM + (half + 1) * 512],
                                    start=False,
                                    stop=(hp == 3),
                                )
                            ob_sb = obpool.tile([P, 512], f32, tag="ob")
                            nc.vector.tensor_copy(ob_sb[:, :], ops[:, :])
                            nc.sync.dma_start(
                                out=out_t[orow : orow + P, half * 512 : (half + 1) * 512],
                                in_=ob_sb[:, :],
                            )

    nc.compile()
    return nc


def prep_inputs(x, rel_pos_bias, Wq, Wkv, Wo, bo):
    """Build the 8 per-core input maps (host-side sharding/marshalling)."""
    x = np.asarray(x, dtype=np.float32)
    rel_pos_bias = np.asarray(rel_pos_bias, dtype=np.float32)
    Wq = np.ascontiguousarray(np.asarray(Wq, dtype=np.float32))
    Wkv = np.asarray(Wkv, dtype=np.float32)
    Wo = np.ascontiguousarray(np.asarray(Wo, dtype=np.float32))
    bo = np.asarray(bo, dtype=np.float32).reshape(1, DIM)
    Wk = np.ascontiguousarray(Wkv[:, :DH])
    Wv = np.ascontiguousarray(Wkv[:, DH:])
    ident = np.eye(P, dtype=BF16)

    ji = np.arange(N)  # global key index
    in_maps = []
    for c in range(NCORES):
        tiles = _q_tiles(c)
        # own tokens, order [b0A, b0B, b1A, b1B]
        xs = [x[b, t * P : (t + 1) * P, :] for b in range(B) for t in tiles]
        xT = np.ascontiguousarray(np.concatenate(xs, axis=0).T)  # [1024, 512]

        biasT = np.full((B, NPAIR, P, HEADS, P), 8.0 * NEG, dtype=np.float32)
        for b in range(B):
            for sl, (t, ext) in enumerate(zip(tiles, (EXTA, EXTB))):
                qg = t * P + np.arange(P)  # global q index [128]
                nj = ext * P
                # [h, q, j] -> [jt, j, h, q]
                blk = rel_pos_bias[:, t * P : (t + 1) * P, :nj]
                blk = 8.0 * blk.reshape(HEADS, P, ext, P).transpose(2, 3, 0, 1)
                m = ji[:nj, None] > qg[None, :]  # [j, q] masked
                blk = np.where(
                    m.reshape(ext, P, 1, P).repeat(HEADS, axis=2)[:, :, :HEADS, :],
                    8.0 * NEG,
                    blk,
                )
                base = 0 if sl == 0 else EXTA
                biasT[b, base : base + ext] = blk
        in_maps.append(
            {
                "xT": xT,
                "Wq": Wq,
                "Wk": Wk,
                "Wv": Wv,
                "Wo": Wo.astype(BF16),
                "bo": bo,
                "ident": ident,
                "ones_r": np.ones((1, P), np.float32),
                "ones_b": np.ones((P, 1), dtype=BF16),
                "biasT": biasT.astype(BF16),
            }
        )
    return in_maps


def assemble(outs):
    """outs: list of 8 [512, 1024] arrays -> full [2, 2048, 1024]."""
    full = np.empty((B, N, DIM), dtype=np.float32)
    for c in range(NCORES):
        o = np.asarray(outs[c])
        for b in range(B):
            for sl, t in enumerate(_q_tiles(c)):
                full[b, t * P : (t + 1) * P, :] = o[(2 * b + sl) * P : (2 * b + sl + 1) * P]
    return full


def kernel(**inputs):
    from concourse.bass_utils import run_bass_kernel_spmd

    if "nc" not in _CACHE:
        _CACHE["nc"] = build_graph()
    nc = _CACHE["nc"]
    in_maps = prep_inputs(
        inputs["x"], inputs["rel_pos_bias"], inputs["Wq"], inputs["Wkv"],
        inputs["Wo"], inputs["bo"],
    )
    res = run_bass_kernel_spmd(
        nc, in_maps, core_ids=list(range(NCORES)),
        trace=bool(int(os.environ.get("KERNEL_TRACE", "0"))),
    )
    _CACHE["last_results"] = res
    return assemble([r["out"] for r in res.results])



# revision 2
# speedup vs baseline: 1.2282x; 1.2282x over previous
"""Trainium2 8-core SPMD kernel for MQA attention with relative position bias.

v5: fully independent cores, no collective, host-prechunked DMA layouts.
  - queries sharded: core c owns q-tiles {c, 15-c} per batch (512 tok/core),
    attention padded to a uniform 8+16 (slot,j-tile) pairs per batch.
  - k/v REPLICATED: each core projects k/v for all 4096 tokens (bf16 x).
  - every DMA moves a [128, contiguous] block (host pre-chunks x, weights
    and bias into partition-major layouts): one run per partition keeps
    the HWDGE issue slices short (multi-run APs measured 6-16us each and
    blocked the engine stream).
  - x and weights + output stream on the Sync HWDGE ring; the 12.6MB
    bias streams as 1MB 4-pair chunks on the Scalar ring.
  - bias injected into the scores PSUM with PE identity-matmuls.
  - scores run as TWO CONCURRENT K=64 row-tiles (heads 0-3 in rows 0:63,
    heads 4-7 in rows 64:127, separate PSUM banks).
  - softmax 1/l = exp(-ln(l)) on ACT; the activation-table map is patched
    so Exp resolves to the natural_log_exp_and_others set (otherwise the
    Ln<->Exp alternation reloads ACT tables twice per slot, ~2.7us each).
"""

import os
import sys

import numpy as np

sys.path.insert(0, "/opt/trn_rl_repo")

import ml_dtypes

BF16 = ml_dtypes.bfloat16

# ---- problem constants (hardcoded per the harness contract) ----
B = 2
N = 2048
DIM = 1024
HEADS = 8
DH = 64
INNER = HEADS * DH  # 512
P = 128
NT = N // P  # 16 q/k tiles per batch
EXTA, EXTB = 8, 16  # j-tile extents for slot A (q-tile c) / slot B (15-c)
NPAIR = EXTA + EXTB  # 24 (slot, j-tile) pairs per batch per core
NCHUNK = NPAIR // 4  # 6 bias chunks per batch (4 pairs each)
NCORES = 8
TOK_OWN = 4 * P  # 512 own tokens per core
BN = B * N  # 4096 total tokens
NTC = BN // 512  # 8 token-chunks for the kv projection
NEG = -1.0e30  # masked logit (pre-scale), exp -> 0

_CACHE = {}


def _q_tiles(c):
    return [c, NT - 1 - c]


def _patch_act_tables():
    """Make Exp resolve to the set that also holds Ln (set 6,
    natural_log_exp_and_others) so the per-slot Ln -> Exp normalize does
    not force ACT table reloads. Set ids stay positional (walrus maps the
    id into act_info.json), only the selection input is filtered."""
    import functools

    import concourse.bacc as bacc_mod
    import concourse.hw_specs as hw_specs_mod
    import concourse.mybir as mybir

    if getattr(hw_specs_mod, "_exp_set_patched", False):
        return
    orig = hw_specs_mod.get_activation_tables

    @functools.cache
    def patched(arch):
        tabs = orig(arch)
        exp = mybir.ActivationFunctionType.Exp
        out = {}
        for name, fns in tabs.items():
            if exp in fns and "natural_log" not in name:
                fns = fns - {exp}
            out[name] = fns
        return out

    hw_specs_mod.get_activation_tables = patched
    hw_specs_mod._exp_set_patched = True
    if hasattr(bacc_mod, "get_activation_tables"):
        bacc_mod.get_activation_tables = patched


def build_graph():
    import concourse.bass as bass
    import concourse.bacc as bacc
    import concourse.mybir as mybir
    import concourse.tile as tile

    _patch_act_tables()

    dt = mybir.dt
    f32, f32r, bf16 = dt.float32, dt.float32r, dt.bfloat16
    AF = mybir.ActivationFunctionType

    nc = bacc.Bacc(None, target_bir_lowering=False)

    # ---- I/O (all pre-chunked on host to [128, contiguous] blocks) ----
    xTa_t = nc.dram_tensor("xTaC", [NTC, P, 8 * 512], bf16, kind="ExternalInput")
    xTo_t = nc.dram_tensor("xToC", [P, 8 * TOK_OWN], bf16, kind="ExternalInput")
    Wq_t = nc.dram_tensor("WqC", [P, 8 * INNER], bf16, kind="ExternalInput")
    Wkv_t = nc.dram_tensor("WkvC", [P, 8 * 2 * DH], bf16, kind="ExternalInput")
    Wo_t = nc.dram_tensor("WoC", [P, 4 * DIM], bf16, kind="ExternalInput")
    bo_t = nc.dram_tensor("bo", [1, DIM], f32r, kind="ExternalInput")
    ident_t = nc.dram_tensor("ident", [P, P], bf16, kind="ExternalInput")
    ones_r_t = nc.dram_tensor("ones_r", [1, P], f32r, kind="ExternalInput")
    ones_bn_t = nc.dram_tensor("ones_bn", [1, BN], bf16, kind="ExternalInput")
    # biasC[b, chunk, j, (4 pairs x h x q)]: transposed, masked, x8-scaled
    bias_t = nc.dram_tensor(
        "biasC", [B, NCHUNK, P, 4 * HEADS * P], bf16, kind="ExternalInput"
    )
    out_t = nc.dram_tensor("out", [TOK_OWN, DIM], f32, kind="ExternalOutput")

    with tile.TileContext(nc) as tc:
        with (
            tc.tile_pool(name="const", bufs=1) as cpool,
            tc.tile_pool(name="bias", bufs=3) as bpool,
            tc.tile_pool(name="pt", bufs=3) as ptpool,
            tc.tile_pool(name="at", bufs=2) as atpool,
            tc.tile_pool(name="ob", bufs=2) as obpool,
            tc.tile_pool(name="ps", bufs=2, space="PSUM") as pspool,
        ):
            # ---- constants / weights into SBUF (sync ring) ----
            ident_sb = cpool.tile([P, P], bf16, tag="ident_sb")
            nc.sync.dma_start(out=ident_sb[:], in_=ident_t[:])
            ones128 = cpool.tile([1, P], f32r, tag="ones128")
            nc.sync.dma_start(out=ones128[:], in_=ones_r_t[:])
            bo_sb = cpool.tile([1, DIM], f32r, tag="bo_sb")
            nc.sync.dma_start(out=bo_sb[:], in_=bo_t[:])
            Wkv_sb = cpool.tile([P, 8 * 2 * DH], bf16, tag="Wkv_sb")
            nc.sync.dma_start(out=Wkv_sb[:], in_=Wkv_t[:])
            kT2 = cpool.tile([P, BN], bf16, tag="kT2")
            vT65 = cpool.tile([DH + 1, BN], bf16, tag="vT65")
            nc.scalar.dma_start(out=vT65[DH : DH + 1, :], in_=ones_bn_t[:])
            V65 = cpool.tile([P, B * NT * (DH + 1)], bf16, tag="V65")
            qT_sb = cpool.tile([P, HEADS * TOK_OWN], bf16, tag="qT_sb")

            # full x, one clean 1MB DMA per token-chunk (tc-major SBUF)
            xTa_sb = cpool.tile([P, NTC * 8 * 512], bf16, tag="xTa_sb")

            def xta_dma(tc_i):
                nc.sync.dma_start(
                    out=xTa_sb[:, tc_i * 4096 : (tc_i + 1) * 4096],
                    in_=xTa_t[tc_i],
                )

            xta_dma(0)
            xta_dma(1)
            xTo_sb = cpool.tile([P, 8 * TOK_OWN], bf16, tag="xTo_sb")
            nc.scalar.dma_start(out=xTo_sb[:], in_=xTo_t[:])
            Wq_sb = cpool.tile([P, 8 * INNER], bf16, tag="Wq_sb")
            nc.sync.dma_start(out=Wq_sb[:], in_=Wq_t[:])
            xta_dma(2)
            xta_dma(3)
            Wo_sb = cpool.tile([P, 4 * DIM], bf16, tag="Wo_sb")
            nc.sync.dma_start(out=Wo_sb[:], in_=Wo_t[:])
            for tc_i in range(4, NTC):
                xta_dma(tc_i)

            # ---- k/v projection chunks (512 tokens each) ----
            def kv_chunk(tc_i):
                kvps = pspool.tile([P, 512], f32, tag="sT", name=f"kv{tc_i}")
                for fc in range(8):
                    nc.tensor.matmul(
                        kvps[:, :],
                        Wkv_sb[:, fc * 2 * DH : (fc + 1) * 2 * DH],
                        xTa_sb[:, tc_i * 4096 + fc * 512 : tc_i * 4096 + (fc + 1) * 512],
                        start=(fc == 0),
                        stop=(fc == 7),
                    )
                nc.vector.tensor_copy(
                    kT2[0:DH, tc_i * 512 : (tc_i + 1) * 512], kvps[0:DH, :]
                )
                nc.vector.tensor_copy(
                    kT2[DH:P, tc_i * 512 : (tc_i + 1) * 512],
                    kT2[0:DH, tc_i * 512 : (tc_i + 1) * 512],
                )
                nc.vector.tensor_copy(
                    vT65[0:DH, tc_i * 512 : (tc_i + 1) * 512], kvps[DH:P, :]
                )
                for i in range(4):
                    g = tc_i * 4 + i  # global j-tile index (b*NT + jt)
                    vtp = pspool.tile([P, DH + 1], f32, tag="sT", name=f"vt{g}")
                    nc.tensor.matmul(
                        vtp[:, :],
                        vT65[:, g * P : (g + 1) * P],
                        ident_sb[0 : DH + 1, 0 : DH + 1],
                        start=True,
                        stop=True,
                    )
                    nc.vector.tensor_copy(
                        V65[:, g * (DH + 1) : (g + 1) * (DH + 1)], vtp[:, :]
                    )

            def q_chunk(hp):
                qps = pspool.tile([P, TOK_OWN], f32, tag="sT", name=f"q{hp}")
                for fc in range(8):
                    nc.tensor.matmul(
                        qps[:, :],
                        Wq_sb[:, fc * INNER + hp * P : fc * INNER + (hp + 1) * P],
                        xTo_sb[:, fc * TOK_OWN : (fc + 1) * TOK_OWN],
                        start=(fc == 0),
                        stop=(fc == 7),
                    )
                nc.vector.tensor_copy(
                    qT_sb[0:DH, (2 * hp) * TOK_OWN : (2 * hp + 1) * TOK_OWN],
                    qps[0:DH, :],
                )
                nc.vector.tensor_copy(
                    qT_sb[0:DH, (2 * hp + 1) * TOK_OWN : (2 * hp + 2) * TOK_OWN],
                    qps[DH:P, :],
                )

            kv_chunk(0)
            kv_chunk(1)
            q_chunk(0)
            q_chunk(1)
            kv_chunk(2)
            q_chunk(2)
            q_chunk(3)
            kv_chunk(3)

            # heads 4-7 duplicated into partitions 64:127 (for the
            # concurrent row-tile scores matmul)
            nc.vector.tensor_copy(
                qT_sb[DH:P, 0 : 4 * TOK_OWN], qT_sb[0:DH, 4 * TOK_OWN : 8 * TOK_OWN]
            )

            # batch-1 kv chunks injected between early batch-0 pairs
            late_kv = {8: 4, 12: 5, 16: 6, 20: 7}

            # ---- attention + output projection per (batch, slot) ----
            qT3 = qT_sb[:, :].rearrange("p (h t) -> p h t", h=HEADS)
            pair_ctr = 0
            for b in range(B):
                for sl, ext in ((0, EXTA), (1, EXTB)):
                    qcol = (2 * b + sl) * P  # q columns in qT/attnT order
                    pv = pspool.tile(
                        [P, HEADS * P], f32, tag="pv", name=f"pv{b}{sl}"
                    )
                    for jt in range(ext):
                        if pair_ctr in late_kv:
                            kv_chunk(late_kv[pair_ctr])
                        pair_ctr += 1
                        pair = jt if sl == 0 else EXTA + jt
                        # bias arrives in 1MB chunks of 4 consecutive pairs
                        # (scalar ring; [128, 4096] contiguous per chunk)
                        if pair % 4 == 0:
                            bias_sb = bpool.tile(
                                [P, 4 * HEADS * P], bf16, tag="bias"
                            )
                            nc.scalar.dma_start(
                                out=bias_sb[:], in_=bias_t[b, pair // 4]
                            )
                        boff = (pair % 4) * HEADS * P
                        sT = pspool.tile(
                            [P, HEADS * P], f32, tag="sT", name=f"sT{b}{sl}{jt}"
                        )
                        kcol = (b * NT + jt) * P
                        # bias injection: sT = ident.T @ bias
                        for half in range(2):
                            nc.tensor.matmul(
                                sT[:, half * 512 : (half + 1) * 512],
                                ident_sb[:, :],
                                bias_sb[:, boff + half * 512 : boff + (half + 1) * 512],
                                start=True,
                                stop=False,
                                skip_group_check=True,
                            )
                        # scores: two CONCURRENT K=64 row-tiles (different
                        # PSUM banks; lhsT base_partition picks the row tile)
                        nc.tensor.matmul(
                            sT[:, 0:512],
                            kT2[0:DH, kcol : kcol + P],
                            qT3[0:DH, 0:4, qcol : qcol + P],
                            start=False,
                            stop=True,
                            skip_group_check=True,
                        )
                        nc.tensor.matmul(
                            sT[:, 512:1024],
                            kT2[DH:P, kcol : kcol + P],
                            qT3[DH:P, 0:4, qcol : qcol + P],
                            start=False,
                            stop=True,
                            skip_group_check=True,
                        )
                        # softmax numerator: P^T = exp(sT/8) in bf16
                        pt_sb = ptpool.tile([P, HEADS * P], bf16, tag="pt")
                        nc.scalar.activation(
                            pt_sb[:, :], sT[:, :], AF.Exp, scale=0.125
                        )
                        # PV: pv[0:65, :] += V65.T @ P^T (denominator in row 64)
                        g = (b * NT + jt) * (DH + 1)
                        for half in range(2):
                            nc.tensor.matmul(
                                pv[0 : DH + 1, half * 512 : (half + 1) * 512],
                                V65[:, g : g + DH + 1],
                                pt_sb[:, half * 512 : (half + 1) * 512],
                                start=(jt == 0),
                                stop=(jt == ext - 1),
                                skip_group_check=True,
                            )

                    # ---- normalize: attnT = pv[0:64] * (1/l) ----
                    # 1/l via ACT: exp(-log(l)) — table patch keeps both fns
                    # in the loaded set
                    recip = cpool.tile(
                        [1, HEADS * P], f32, name=f"rc{b}{sl}", tag="recip", bufs=2
                    )
                    lg = cpool.tile(
                        [1, HEADS * P], f32, name=f"lg{b}{sl}", tag="lg", bufs=2
                    )
                    nc.scalar.activation(lg[:, :], pv[DH : DH + 1, :], AF.Ln)
                    nc.scalar.activation(recip[:, :], lg[:, :], AF.Exp, scale=-1.0)
                    bc_sb = ptpool.tile(
                        [DH, HEADS * P], f32, tag="pt", name=f"bc{b}{sl}"
                    )
                    nc.gpsimd.partition_broadcast(bc_sb[:, :], recip[:, :])
                    attnT = atpool.tile([P, HEADS * P], bf16, tag="at")
                    for half in range(2):
                        fs = slice(half * 512, (half + 1) * 512)
                        nc.vector.tensor_mul(attnT[0:DH, fs], pv[0:DH, fs], bc_sb[:, fs])
                    # shifted duplicate: rows 64:128 col g*128 hold head g+1
                    nc.vector.tensor_copy(
                        attnT[DH:P, 0 : 7 * P], attnT[0:DH, P : HEADS * P]
                    )

                    # ---- output projection for this slot's 128 tokens ----
                    orow = (2 * b + sl) * P
                    for half in range(2):
                        fs = slice(half * 512, (half + 1) * 512)
                        ops = pspool.tile(
                            [P, 512], f32, tag="pv", name=f"op{b}{sl}{half}"
                        )
                        nc.tensor.matmul(
                            ops[:, :], ones128[:, :], bo_sb[:, fs], start=True, stop=False
                        )
                        for hp in range(4):
                            nc.tensor.matmul(
                                ops[:, :],
                                attnT[:, 2 * hp * P : (2 * hp + 1) * P],
                                Wo_sb[:, hp * DIM + half * 512 : hp * DIM + (half + 1) * 512],
                                start=False,
                                stop=(hp == 3),
                            )
                        ob_sb = obpool.tile([P, 512], f32, tag="ob")
                        nc.vector.tensor_copy(ob_sb[:, :], ops[:, :])
                        nc.sync.dma_start(
                            out=out_t[orow : orow + P, half * 512 : (half + 1) * 512],
                            in_=ob_sb[:, :],
                        )

    nc.compile()
    return nc


def prep_inputs(x, rel_pos_bias, Wq, Wkv, Wo, bo):
    """Build the 8 per-core input maps (host-side sharding/marshalling)."""
    x = np.asarray(x, dtype=np.float32)
    rel_pos_bias = np.asarray(rel_pos_bias, dtype=np.float32)
    bo = np.asarray(bo, dtype=np.float32).reshape(1, DIM)

    def chunk_pm(w, nf):
        # [nf*128, C] -> [128, nf*C] (partition-major chunks)
        w = np.asarray(w, dtype=np.float32)
        c = w.shape[1]
        return np.ascontiguousarray(
            w.reshape(nf, P, c).transpose(1, 0, 2).reshape(P, nf * c)
        ).astype(BF16)

    WqC = chunk_pm(Wq, 8)
    WkvC = chunk_pm(Wkv, 8)
    WoC = chunk_pm(Wo, 4)
    xT = np.concatenate([x[b].T for b in range(B)], axis=1)  # [1024, 4096]
    # [tc, p, fc*512+t] = xT[fc*128+p, tc*512+t]
    xTaC = np.ascontiguousarray(
        xT.reshape(8, P, 8, 512).transpose(2, 1, 0, 3).reshape(NTC, P, 8 * 512)
    ).astype(BF16)
    ident = np.eye(P, dtype=BF16)
    ones_r = np.ones((1, P), np.float32)
    ones_bn = np.ones((1, BN), dtype=BF16)

    ji = np.arange(N)  # global key index
    in_maps = []
    for c in range(NCORES):
        tiles = _q_tiles(c)
        # own tokens, order [b0A, b0B, b1A, b1B]
        xs = [x[b, t * P : (t + 1) * P, :] for b in range(B) for t in tiles]
        xToC = chunk_pm(np.concatenate(xs, axis=0).T, 8)

        biasT = np.full((B, NPAIR, P, HEADS, P), 8.0 * NEG, dtype=np.float32)
        for b in range(B):
            for sl, (t, ext) in enumerate(zip(tiles, (EXTA, EXTB))):
                qg = t * P + np.arange(P)  # global q index [128]
                nj = ext * P
                # [h, q, j] -> [jt, j, h, q]
                blk = rel_pos_bias[:, t * P : (t + 1) * P, :nj]
                blk = 8.0 * blk.reshape(HEADS, P, ext, P).transpose(2, 3, 0, 1)
                m = ji[:nj, None] > qg[None, :]  # [j, q] masked
                blk = np.where(
                    m.reshape(ext, P, 1, P).repeat(HEADS, axis=2)[:, :, :HEADS, :],
                    8.0 * NEG,
                    blk,
                )
                base = 0 if sl == 0 else EXTA
                biasT[b, base : base + ext] = blk
        # [b, pair, j, h, q] -> [b, chunk, j, (pair%4, h, q)]
        biasC = np.ascontiguousarray(
            biasT.reshape(B, NCHUNK, 4, P, HEADS, P)
            .transpose(0, 1, 3, 2, 4, 5)
            .reshape(B, NCHUNK, P, 4 * HEADS * P)
        ).astype(BF16)
        in_maps.append(
            {
                "xTaC": xTaC,
                "xToC": xToC,
                "WqC": WqC,
                "WkvC": WkvC,
                "WoC": WoC,
                "bo": bo,
                "ident": ident,
                "ones_r": ones_r,
                "ones_bn": ones_bn,
                "biasC": biasC,
            }
        )
    return in_maps


def assemble(outs):
    """outs: list of 8 [512, 1024] arrays -> full [2, 2048, 1024]."""
    full = np.empty((B, N, DIM), dtype=np.float32)
    for c in range(NCORES):
        o = np.asarray(outs[c])
        for b in range(B):
            for sl, t in enumerate(_q_tiles(c)):
                full[b, t * P : (t + 1) * P, :] = o[
                    (2 * b + sl) * P : (2 * b + sl + 1) * P
                ]
    return full


def kernel(**inputs):
    from concourse.bass_utils import run_bass_kernel_spmd

    if "nc" not in _CACHE:
        _CACHE["nc"] = build_graph()
    nc = _CACHE["nc"]
    in_maps = prep_inputs(
        inputs["x"], inputs["rel_pos_bias"], inputs["Wq"], inputs["Wkv"],
        inputs["Wo"], inputs["bo"],
    )
    res = run_bass_kernel_spmd(
        nc, in_maps, core_ids=list(range(NCORES)),
        trace=bool(int(os.environ.get("KERNEL_TRACE", "0"))),
    )
    _CACHE["last_results"] = res
    return assemble([r["out"] for r in res.results])
